# revision 29
# baseline (speedup 1.0000x reference)
"""CapsuleLayer (dynamic routing) Trainium2 Bass kernel, v2.

Full-input contract: kernel(inputs, W) -> [256, 10, 16, 1] f32.
Data-parallel over batch: 8 cores x 32 batches, W replicated.

Math restructuring vs the reference (carried over from v1):
  - routing logits are always b_t = u_hat * V_t with V_t the running sum of
    squashed outputs, so only V[b,n,d] is kept (no [B,NC,IC,DC] logits).
  - pass 1 (uniform softmax) reduces to s1 = 0.1 * sum_i u_hat.

Numerics (measured with a numpy emulation of each rounding site):
  - the routing iteration chaotically amplifies perturbations: a 5e-4
    relative rounding of u_hat / s1 / any pass-2 intermediate lands at
    0.05-0.15 final error (gate is 2e-2). Production, s1 and all of pass 2
    therefore stay exact f32.
  - pass-3 errors hit the output directly (~5x): the entire pass-3
    elementwise chain runs in f16 (measured 2.7e-3 end to end), with the
    fold as an f16 matmul into an f32 PSUM.

v2 structural changes vs v1 (baseline 715us):
  - fold masks are [128,128] (tile(eye(16),(8,8))): the PE fold emits the
    capsule sum already replicated across the 8 partition quadrants, so the
    8-DMA bcast16 step (and its latency on every pass boundary) is gone.
  - pass 3 entirely in f16: 2x DVE TensorTensor, 4x faster fold matmul.
    u16 is re-materialized per chunk (DVE 2x_2p copy) to keep SBUF low.
  - squash restructured: Square and Sqrt(+eps as fused bias) on ScalarE,
    reciprocal_approx_fast/accurate instead of the iterative chain.
  - s1 partially folded on the PE (f32 mask matmul, PSUM accumulation),
    remainder on the DVE pair-tree - split set by S1_PE_CHUNKS.
  - engine knobs (TR_POOL / Y2_POOL / C2_POOL masks) move per-chunk ops to
    the otherwise idle GpSimd engine.
"""

import os
import sys

import numpy as np

sys.path.insert(0, "/opt/trn_rl_repo")

B, IC, ID = 256, 1152, 8
NC, DC = 10, 16
NCORES = 8
BC = B // NCORES            # 32 batches per core
SB = 2                      # sub-batches per core
BB = BC // SB               # 16 batches per sub-batch
IPK = 8                     # input capsules packed per group
G2 = IC // IPK              # 144 groups
K2 = IPK * ID               # 64 contraction rows
ND = NC * DC                # 160
FREE2 = G2 * ND             # 23040
CH = 12                     # chunk size in groups (production & routing)
NCH = G2 // CH              # 12 chunks
PSUM_GRP = 3                # groups per PSUM bank tile (3*160*4B < 2KB)
EPS = 1e-7

_CACHE = {}

# --- engine-assignment knobs (tuned against CoreSim, validated on HW) ----
# s1: how many of the 12 chunks fold on the PE (f32 mask matmul); the rest
# use the pair-tree (on the engine given by s1_pool).
S1_PE_CHUNKS = 0
# per-chunk offloads to GpSimd (Pool): chunk c uses Pool when
# (c % den) < num.
TR2_POOL = (1, 1)   # denominator-reduce pair-tree on Pool for pass 2
TR3_POOL = (1, 1)   # same for pass 3
Y2_POOL = (1, 1)    # y = e*u on Pool (contiguous TT)
C2_POOL = (0, 1)    # y *= rv broadcast on Pool
X2_POOL = (0, 1)    # x = u*V broadcast on Pool


def _sel(knob, c):
    num, den = knob
    return (c % den) < num


def _build_nc(reps=1, knobs=None, debug=False):
    import contextlib

    import concourse.bacc as bacc
    import concourse.mybir as mybir
    import concourse.tile as tile

    kn = {
        "s1_pe": S1_PE_CHUNKS,
        "tr2": TR2_POOL, "tr3": TR3_POOL,
        "y2": Y2_POOL, "c2": C2_POOL, "x2": X2_POOL,
        "y3": (0, 1), "c3": (1, 3),   # pass-3 f16 TTs on Pool
        "s1_pool": (1, 1),            # s1-tree chunks on Pool
        "u16_act": False,             # u16 re-materialization on ScalarE
        "xbufs": 3, "u16bufs": 4, "qbufs": 2,
        "fast_recip": True,           # 51-ULP recip everywhere (vs 2-ULP)
    }
    if knobs:
        kn.update(knobs)

    F32 = mybir.dt.float32
    F16 = mybir.dt.float16
    ALU = mybir.AluOpType
    ACTF = mybir.ActivationFunctionType

    nc = bacc.Bacc()
    lt_d = nc.dram_tensor("lt", [K2, G2 * 128], F32, kind="ExternalInput")
    xt_d = nc.dram_tensor("xt", [K2, G2 * BB], F32, kind="ExternalInput")
    mlt_d = nc.dram_tensor("mlt", [K2, 128], F32, kind="ExternalInput")
    wr_d = nc.dram_tensor("wr", [K2, FREE2], F32, kind="ExternalInput")
    m128_d = nc.dram_tensor("m128", [128, 128], F32, kind="ExternalInput")
    out_d = nc.dram_tensor("out", [BC, ND], F32, kind="ExternalOutput")
    if debug:
        dbg_s1 = nc.dram_tensor("dbg_s1", [SB * 128, ND], F32,
                                kind="ExternalOutput")
        dbg_v1 = nc.dram_tensor("dbg_v1", [SB * 128, ND], F32,
                                kind="ExternalOutput")
        dbg_s2 = nc.dram_tensor("dbg_s2", [SB * 128, ND], F32,
                                kind="ExternalOutput")
        dbg_u = nc.dram_tensor("dbg_u", [128, CH * ND], F32,
                               kind="ExternalOutput")

    with tile.TileContext(nc) as tc:
        with (
            tc.tile_pool(name="const", bufs=1) as cpool,
            tc.tile_pool(name="sq", bufs=kn["qbufs"]) as qpool,
            tc.tile_pool(name="uhp", bufs=NCH) as uhp,
            tc.tile_pool(name="psw", bufs=2, space="PSUM") as swpool,
            tc.tile_pool(name="pss1", bufs=1, space="PSUM") as s1psum,
            tc.tile_pool(name="ltp", bufs=2) as ltp,
            tc.tile_pool(name="wrp", bufs=2) as wrp,
            tc.tile_pool(name="pprod", bufs=4, space="PSUM") as pprod,
            tc.tile_pool(name="x", bufs=kn["xbufs"]) as xpool,
            tc.tile_pool(name="u16", bufs=kn["u16bufs"]) as u16p,
            tc.tile_pool(name="dn", bufs=3) as dnpool,
            tc.tile_pool(name="dntree", bufs=2) as dtpool,
            tc.tile_pool(name="s1t", bufs=1) as s1pool,
        ):
            m128_t = cpool.tile([128, 128], F32)
            nc.sync.dma_start(m128_t[:], m128_d[:])
            mlt_t = cpool.tile([K2, 128], F32)
            nc.sync.dma_start(mlt_t[:], mlt_d[:])
            m128_h = cpool.tile([128, 128], F16)
            nc.vector.tensor_copy(m128_h[:], m128_t[:])
            eps_t = cpool.tile([128, 1], F32)
            nc.vector.memset(eps_t[:], EPS)
            # pass-3 exp bias: e' = exp(x - 1) keeps e' and 1/sum(e') far
            # from f16 limits (softmax is shift-invariant, c unchanged).
            nb_t = cpool.tile([128, 1], F32)
            nc.vector.memset(nb_t[:], -1.0)

            rep_ctx = (
                tc.For_i(0, reps, 1) if reps > 1 else contextlib.nullcontext()
            )

            def squash(s, vt, accurate):
                # vt = s^3 / ((1+s^2) sqrt(s^2+eps)); ScalarE: Square + Sqrt
                # (eps folded into the Sqrt bias); DVE: the rest.
                sq = qpool.tile([128, ND], F32, tag="sq_sq")
                nc.scalar.activation(sq[:], s[:], ACTF.Square)
                a = qpool.tile([128, ND], F32, tag="sq_a")
                nc.scalar.activation(a[:], sq[:], ACTF.Sqrt, bias=eps_t[:])
                d2 = qpool.tile([128, ND], F32, tag="sq_d2")
                nc.vector.scalar_tensor_tensor(
                    d2[:], sq[:], 1.0, a[:], op0=ALU.add, op1=ALU.mult
                )
                r = qpool.tile([128, ND], F32, tag="sq_r")
                if accurate and not kn["fast_recip"]:
                    r_s = qpool.tile([128, ND], F32, tag="sq_rs")
                    nc.vector.reciprocal_approx_accurate(r[:], d2[:], r_s[:])
                else:
                    nc.vector.reciprocal_approx_fast(r[:], d2[:])
                t1 = qpool.tile([128, ND], F32, tag="sq_t1")
                nc.vector.tensor_mul(t1[:], s[:], sq[:])
                nc.vector.tensor_mul(vt[:], t1[:], r[:])

            def denom_tree(eng, e_t, dn, F):
                # dn[p,(g,d)] = sum_n e[p,(g,n,d)] via pair-tree adds
                # (gpsimd has no free-axis tensor_reduce).
                # in-place shrinking tree inside one [CH,5,DC] scratch tile
                e4 = e_t[:].rearrange("p (g n d) -> p g n d", n=NC, d=DC)
                t5 = dtpool.tile([128, CH * 5 * DC], F, tag="t5")
                t54 = t5[:].rearrange("p (g n d) -> p g n d", n=5, d=DC)
                eng.tensor_tensor(
                    t54, e4[:, :, 0:5, :], e4[:, :, 5:10, :], ALU.add
                )
                eng.tensor_tensor(
                    t54[:, :, 0:2, :], t54[:, :, 0:2, :],
                    t54[:, :, 2:4, :], ALU.add
                )
                eng.tensor_tensor(
                    t54[:, :, 0:1, :], t54[:, :, 0:1, :],
                    t54[:, :, 1:2, :], ALU.add
                )
                dn4 = dn[:].rearrange("p (g o d) -> p g o d", o=1, d=DC)
                eng.tensor_tensor(
                    dn4, t54[:, :, 0:1, :], t54[:, :, 4:5, :], ALU.add
                )

            with rep_ctx:
              for s_i in range(SB):
                V = cpool.tile([128, ND], F32, tag=f"V{s_i}")
                V16 = cpool.tile([128, ND], F16, tag=f"V16{s_i}")
                # ---------- production: u_hat + s1 fold ----------
                uch = []
                s1acc = cpool.tile([128, ND], F32, tag="s1acc")
                ps1 = s1psum.tile([128, PSUM_GRP * ND], F32, tag="ps1")
                n_s1pe = 0
                # +1: the DVE-tree partial (s1acc) is partition-folded into
                # the same PSUM region by one extra matmul after the loop.
                has_tree = kn["s1_pe"] < NCH
                n_s1pe_tot = kn["s1_pe"] * (CH // PSUM_GRP) + int(has_tree)
                first_tree = True
                for c in range(NCH):
                    g0 = c * CH
                    ltt = ltp.tile([K2, CH * 128], F32, tag="ltt")
                    if s_i == 0:
                        xtt = ltp.tile([K2, CH * BB], F32, tag="xtt")
                        nc.sync.dma_start(
                            xtt[:], xt_d[:, g0 * BB:(g0 + CH) * BB]
                        )
                        ltt4 = ltt[:].rearrange(
                            "p (g i b) -> p g i b", i=IPK, b=BB
                        )
                        xt_b = (
                            xtt[:]
                            .rearrange("p (g b) -> p g b", b=BB)
                            .unsqueeze(2)
                            .broadcast_to([K2, CH, IPK, BB])
                        )
                        ml_b = (
                            mlt_t[:]
                            .rearrange("p (i b) -> p i b", b=BB)
                            .unsqueeze(1)
                            .broadcast_to([K2, CH, IPK, BB])
                        )
                        nc.gpsimd.tensor_tensor(ltt4, xt_b, ml_b, ALU.mult)
                    else:
                        nc.sync.dma_start(
                            ltt[:],
                            lt_d[:, g0 * 128:(g0 + CH) * 128],
                        )
                    wrt = wrp.tile([K2, CH * ND], F32)
                    nc.sync.dma_start(
                        wrt[:], wr_d[:, g0 * ND:(g0 + CH) * ND]
                    )
                    u = uhp.tile([128, CH * ND], F32, tag="uh")
                    uch.append(u)
                    for t3 in range(CH // PSUM_GRP):
                        pt = pprod.tile([128, PSUM_GRP * ND], F32)
                        for j in range(PSUM_GRP):
                            gl = t3 * PSUM_GRP + j
                            nc.tensor.matmul(
                                pt[:, j * ND:(j + 1) * ND],
                                ltt[:, gl * 128:(gl + 1) * 128],
                                wrt[:, gl * ND:(gl + 1) * ND],
                                start=True,
                                stop=True,
                            )
                        lo = t3 * PSUM_GRP * ND
                        hi = (t3 + 1) * PSUM_GRP * ND
                        nc.scalar.copy(u[:, lo:hi], pt[:])
                        # s1 PE share: fold these 3 groups into the s1 PSUM
                        if c < kn["s1_pe"]:
                            nc.tensor.matmul(
                                ps1[:],
                                m128_t[:],
                                u[:, lo:hi],
                                start=(n_s1pe == 0),
                                stop=(n_s1pe == n_s1pe_tot - 1),
                            )
                            n_s1pe += 1
                    if c >= kn["s1_pe"]:
                        # pair-tree share (DVE, or Pool per knob)
                        s1e = (nc.gpsimd if _sel(kn["s1_pool"], c)
                               else nc.vector)
                        u3c = u[:].rearrange("p (g nd) -> p g nd", nd=ND)
                        t6 = s1pool.tile([128, 6 * ND], F32, tag="s1a")
                        t63 = t6[:].rearrange("p (g nd) -> p g nd", nd=ND)
                        s1e.tensor_tensor(
                            t63, u3c[:, 0:12:2, :], u3c[:, 1:12:2, :],
                            ALU.add
                        )
                        t3_ = s1pool.tile([128, 3 * ND], F32, tag="s1b")
                        t33 = t3_[:].rearrange("p (g nd) -> p g nd", nd=ND)
                        s1e.tensor_tensor(
                            t33, t63[:, 0:6:2, :], t63[:, 1:6:2, :], ALU.add
                        )
                        sp = s1pool.tile([128, ND], F32, tag="s1p")
                        s1e.tensor_tensor(
                            sp[:], t33[:, 0, :], t33[:, 1, :], ALU.add
                        )
                        s1e.tensor_tensor(sp[:], sp[:], t33[:, 2, :], ALU.add)
                        if first_tree:
                            nc.vector.tensor_copy(s1acc[:], sp[:])
                            first_tree = False
                        else:
                            nc.vector.tensor_add(s1acc[:], s1acc[:], sp[:])
                # fold the DVE-tree partial over partitions into the PSUM
                if has_tree:
                    nc.tensor.matmul(
                        ps1[:, 0:ND], m128_t[:], s1acc[:],
                        start=(n_s1pe == 0), stop=True,
                    )
                    n_s1pe += 1
                # collapse: s1 = 0.1 * (blk0 + blk1 + blk2)
                s1 = qpool.tile([128, ND], F32, tag="c3_s")
                if kn["s1_pe"] > 0:
                    cw1 = qpool.tile([128, PSUM_GRP * ND], F32, tag="c3_w")
                    nc.scalar.activation(cw1[:], ps1[:], ACTF.Copy, scale=0.1)
                    s3a = qpool.tile([128, ND], F32, tag="c3_a")
                    nc.vector.tensor_add(
                        s3a[:], cw1[:, 0:ND], cw1[:, ND:2 * ND]
                    )
                    nc.vector.tensor_add(s1[:], s3a[:], cw1[:, 2 * ND:3 * ND])
                else:
                    nc.scalar.activation(s1[:], ps1[:, 0:ND], ACTF.Copy,
                                         scale=0.1)
                if debug:
                    nc.sync.dma_start(
                        dbg_s1[s_i * 128:(s_i + 1) * 128, :], s1[:]
                    )
                    if s_i == 0:
                        nc.sync.dma_start(dbg_u[:], uch[0][:])
                squash(s1, V, accurate=True)
                if debug:
                    nc.sync.dma_start(
                        dbg_v1[s_i * 128:(s_i + 1) * 128, :], V[:]
                    )

                # ---------- routing passes 2 and 3 ----------
                for t in (2, 3):
                    f16_pass = (t == 3)
                    F = F16 if f16_pass else F32
                    ps_w = swpool.tile([BB * IPK, PSUM_GRP * ND], F32,
                                       tag="psw")
                    n_fold = 0
                    for c in range(NCH):
                        u = uch[c]
                        if f16_pass:
                            u16 = u16p.tile([128, CH * ND], F16)
                            if kn["u16_act"]:
                                nc.scalar.copy(u16[:], u[:])
                            else:
                                nc.vector.tensor_copy(u16[:], u[:])
                            usrc = u16
                        else:
                            usrc = u
                        x = xpool.tile([128, CH * ND], F, tag=f"x{t}")
                        x3 = x[:].rearrange("p (g nd) -> p g nd", nd=ND)
                        u3 = usrc[:].rearrange("p (g nd) -> p g nd", nd=ND)
                        vsrc = V16 if f16_pass else V
                        vb_b = vsrc[:].unsqueeze(1).broadcast_to(
                            [128, CH, ND]
                        )
                        x_eng = (nc.gpsimd
                                 if (not f16_pass and _sel(kn["x2"], c))
                                 else nc.vector)
                        x_eng.tensor_tensor(x3, u3, vb_b, ALU.mult)
                        if f16_pass:
                            nc.scalar.activation(x[:], x[:], ACTF.Exp,
                                                 bias=nb_t[:])
                        else:
                            nc.scalar.activation(x[:], x[:], ACTF.Exp)
                        # denominator (pair-tree; engine per knob)
                        dn = dnpool.tile([128, CH * DC], F32, tag="dn")
                        tr_eng = (nc.gpsimd
                                  if _sel(kn[f"tr{t}"], c) else nc.vector)
                        if tr_eng is nc.vector:
                            x4 = x[:].rearrange(
                                "p (g n d) -> p g n d", n=NC, d=DC
                            )
                            dn4 = dn[:].rearrange(
                                "p (g o d) -> p g o d", o=1, d=DC
                            )
                            nc.vector.tensor_reduce(
                                dn4,
                                x4.transpose([0, 1, 3, 2]),
                                axis=mybir.AxisListType.X,
                                op=ALU.add,
                            )
                        else:
                            denom_tree(nc.gpsimd, x, dn, F32)
                        rv = dnpool.tile([128, CH * DC], F32, tag="rv")
                        if t == 2:
                            if kn["fast_recip"]:
                                nc.vector.reciprocal_approx_fast(
                                    rv[:], dn[:]
                                )
                            else:
                                rv_s = dnpool.tile([128, CH * DC], F32,
                                                   tag="rvs")
                                nc.vector.reciprocal_approx_accurate(
                                    rv[:], dn[:], rv_s[:]
                                )
                            rvt = rv
                        else:
                            nc.vector.reciprocal_approx_fast(rv[:], dn[:])
                            rv16 = dnpool.tile([128, CH * DC], F16,
                                               tag="rv16")
                            nc.vector.tensor_copy(rv16[:], rv[:])
                            rvt = rv16
                        rv_b = (
                            rvt[:]
                            .rearrange("p (g d) -> p g d", d=DC)
                            .unsqueeze(2)
                            .broadcast_to([128, CH, NC, DC])
                        )
                        x4 = x[:].rearrange(
                            "p (g n d) -> p g n d", n=NC, d=DC
                        )
                        y_kn = kn["y2"] if not f16_pass else kn["y3"]
                        y_eng = nc.gpsimd if _sel(y_kn, c) else nc.vector
                        c_kn = kn["c2"] if not f16_pass else kn["c3"]
                        c_eng = nc.gpsimd if _sel(c_kn, c) else nc.vector
                        if f16_pass:
                            # f16: c = e*rv first (c<=1 keeps the chain in
                            # f16 range; e*u would overflow), then y = c*u.
                            c_eng.tensor_tensor(x4, x4, rv_b, ALU.mult)
                            y_eng.tensor_tensor(
                                x[:], x[:], usrc[:], ALU.mult
                            )
                        else:
                            # f32: y = e*u then y *= 1/dn
                            y_eng.tensor_tensor(
                                x[:], x[:], usrc[:], ALU.mult
                            )
                            c_eng.tensor_tensor(x4, x4, rv_b, ALU.mult)
                        mk = m128_h if f16_pass else m128_t
                        for j3 in range(CH // PSUM_GRP):
                            nc.tensor.matmul(
                                ps_w[:],
                                mk[:],
                                x[:, j3 * PSUM_GRP * ND:
                                  (j3 + 1) * PSUM_GRP * ND],
                                start=(n_fold == 0),
                                stop=(n_fold == G2 // PSUM_GRP - 1),
                            )
                            n_fold += 1
                    # collapse [128, 480] PSUM -> s [128, 160]
                    cw = qpool.tile([128, PSUM_GRP * ND], F32, tag="c3_w")
                    nc.scalar.copy(cw[:], ps_w[:])
                    s3 = qpool.tile([128, ND], F32, tag="c3_a")
                    nc.vector.tensor_add(s3[:], cw[:, 0:ND], cw[:, ND:2 * ND])
                    s_t = qpool.tile([128, ND], F32, tag="c3_s")
                    nc.vector.tensor_add(s_t[:], s3[:], cw[:, 2 * ND:3 * ND])
                    if debug and t == 2:
                        nc.sync.dma_start(
                            dbg_s2[s_i * 128:(s_i + 1) * 128, :], s_t[:]
                        )
                    vt = qpool.tile([128, ND], F32, tag="vt")
                    squash(s_t, vt, accurate=(t == 2))
                    if t == 2:
                        nc.vector.tensor_add(V[:], V[:], vt[:])
                        nc.vector.tensor_copy(V16[:], V[:])
                    else:
                        nc.sync.dma_start(
                            out_d[s_i * BB:(s_i + 1) * BB, :],
                            vt[0:BB, :],
                        )
    nc.finalize()
    return nc


def _host_pack(inputs, W):
    """Build per-core LT, shared WR and masks, all f32."""
    inputs = np.ascontiguousarray(inputs, dtype=np.float32)
    W = np.ascontiguousarray(W, dtype=np.float32)

    W6 = W.reshape(NC, G2, IPK, DC, ID)
    wr = np.ascontiguousarray(
        W6.transpose(2, 4, 1, 0, 3).reshape(K2, FREE2)
    )

    m128 = np.ascontiguousarray(
        np.tile(np.eye(BB, dtype=np.float32), (IPK, IPK))
    )
    mlt = np.zeros((K2, 128), dtype=np.float32)
    for i8 in range(IPK):
        mlt[i8 * ID:(i8 + 1) * ID, i8 * BB:(i8 + 1) * BB] = 1.0

    lts, xts = [], []
    for core in range(NCORES):
        xc = inputs[core * BC:(core + 1) * BC]              # [BC, IC, ID]
        x6 = xc.reshape(SB, BB, G2, IPK, ID)                # [s, b, g, i8, k]
        lt = np.zeros((K2, G2, 128), dtype=np.float32)
        for i8 in range(IPK):
            lt[i8 * ID:(i8 + 1) * ID, :, i8 * BB:(i8 + 1) * BB] = (
                x6[1, :, :, i8, :].transpose(2, 1, 0)       # [k, g, b]
            )
        lts.append(np.ascontiguousarray(lt.reshape(K2, G2 * 128)))
        xt = np.zeros((K2, G2, BB), dtype=np.float32)
        for i8 in range(IPK):
            xt[i8 * ID:(i8 + 1) * ID] = x6[0, :, :, i8, :].transpose(2, 1, 0)
        xts.append(np.ascontiguousarray(xt.reshape(K2, G2 * BB)))
    return lts, xts, wr, m128, mlt


def kernel(inputs, W):
    from concourse.bass_utils import run_bass_kernel_spmd

    if "nc" not in _CACHE:
        _CACHE["nc"] = _build_nc()
    nc = _CACHE["nc"]

    lts, xts, wr, m128, mlt = _host_pack(np.asarray(inputs), np.asarray(W))
    in_maps = [
        {"lt": lts[c], "xt": xts[c], "wr": wr, "m128": m128, "mlt": mlt}
        for c in range(NCORES)
    ]
    res = run_bass_kernel_spmd(nc, in_maps, core_ids=list(range(NCORES)))
    outs = [
        np.asarray(res.results[c]["out"]).reshape(BC, NC, DC, 1)
        for c in range(NCORES)
    ]
    return np.concatenate(outs, axis=0).astype(np.float32)


if __name__ == "__main__":
    rng = np.random.default_rng(0)
    x = rng.standard_normal((B, IC, ID), dtype=np.float32)
    w = rng.standard_normal((NC, IC, DC, ID), dtype=np.float32) * 0.1
    out = kernel(x, w)
    print(out.shape, out.dtype)
